# revision 10
# baseline (speedup 1.0000x reference)
"""Distributed single-head attention block for trn2 (8 NeuronCores), fp8.

reference:
    q = x @ Wq.T + bq ; k = x @ Wk.T + bk ; v = x @ Wv.T + bv
    out = x + softmax(q @ k.T / sqrt(D)) @ v       x: [4, 2048, 1024]

Sharding: 8 cores = 4 batches x 2 query-halves. Core c owns batch c//2 and
query rows [h*1024, (h+1)*1024) with h = c%2. Each core recomputes K for
its whole batch (duplicated across the pair — cheaper than the serialized
CC-stream hop a K-exchange costs); V is projected for the own half only
and exchanged via pairwise AllGather, which hides under scoresT. A dummy
128B AllGather issued at kernel start prefetches the ~20us CC rendezvous
barrier so the real exchange starts immediately.

All matmuls run fp8e4 with DoubleRow perf mode (2x PE throughput):
weights are pre-scaled by 16 on the host so W*16 sits in the fp8 normal
range with |q,k,v| < 128 (fp8e4 max 240); the 16*16 factor on scores
folds into the exp scale, and the 16 on V folds into the softmax
reciprocal via a 16-valued ones-vector in the denominator matmul.

Scores are computed TRANSPOSED (scoresT[k,q] = K@Q^T) so the exp output
is already P^T, the lhsT the attention matmul needs — no PE transposes.
Softmax denominators (a partition-dim sum of P^T) come from a tiny
ones-vector matmul. exp uses a -4 bias (cancels between numerator and
denominator; scores reach ~7.4) to keep P far below fp8e4's 240 max.
K's bias cancels in softmax; V's bias is folded into the residual on
the host (attention weights sum to 1).
"""

import numpy as np

B, S, D = 4, 2048, 1024
SQ = S // 2  # query rows owned per core
NCORES = 8
DC = D // 128  # contraction chunks over embed
EC = D // 128  # output embed chunks
SC = S // 128  # key chunks
QT = SQ // 128  # query tiles per core
NPAIR = DC // 2  # DoubleRow pairs per 1024-deep contraction

WSCALE = 16.0  # host pre-scale on Wq/Wk/Wv (and bq); keeps |q,k,v| < 128
SCORE_SCALE = 1.0 / (np.sqrt(D) * WSCALE * WSCALE)  # 1/8192
EXP_SHIFT = 4.0  # exp(s - 4): scores reach ~7.4, so max P ~ e^3.5 << 240

_cache = {}


def _build():
    import concourse.bass as bass
    import concourse.tile as tile
    from concourse import bacc, mybir

    f32 = mybir.dt.float32
    bf16 = mybir.dt.bfloat16
    f8 = mybir.dt.float8e4
    Alu = mybir.AluOpType
    Act = mybir.ActivationFunctionType
    DR = mybir.MatmulPerfMode.DoubleRow

    nc = bacc.Bacc(None, target_bir_lowering=False, debug=False)

    xT_d = nc.declare_dram_parameter("xT", [128, DC, S], f8, isOutput=False)
    xqT_d = nc.declare_dram_parameter("xqT", [128, DC, SQ], f8, isOutput=False)
    wq_d = nc.declare_dram_parameter("wqT", [128, DC, D], f8, isOutput=False)
    wk_d = nc.declare_dram_parameter("wkT", [128, DC, D], f8, isOutput=False)
    wv_d = nc.declare_dram_parameter("wvT", [128, DC, D], f8, isOutput=False)
    bq_d = nc.declare_dram_parameter("bq", [D], f32, isOutput=False)
    xq_d = nc.declare_dram_parameter("xq", [128, QT, D], f32, isOutput=False)
    out_d = nc.declare_dram_parameter("out", [SQ, D], f32, isOutput=True)

    vx_in = nc.dram_tensor("vx_in", [QT, 128, D], f8)
    vx_out = nc.dram_tensor("vx_out", [2, QT, 128, D], f8)
    GROUPS = [[0, 1], [2, 3], [4, 5], [6, 7]]

    with tile.TileContext(nc) as tc:
        with tc.tile_pool(name="pers", bufs=1) as pers:
            qT8 = pers.tile([128, EC, SQ], f8, tag="qT8")
            kT8 = pers.tile([128, EC, S], f8, tag="kT8")
            v8 = pers.tile([128, SC, D], f8, tag="v8")
            PT8 = pers.tile([128, SC, SQ], f8, tag="PT8")
            ones2 = pers.tile([128, 2, 1], f8, tag="ones2")
            bq_sb = pers.tile([128, EC], f32, tag="bq")
            nbias = pers.tile([128, 1], f32, tag="nbias")
            nc.vector.memset(ones2, WSCALE)
            nc.vector.memset(nbias, -float(EXP_SHIFT))

            # PE warmup: dummy matmuls while the first input DMAs land, so
            # the HAM clock gate is already ramped when real work starts.
            warm_sb = pers.tile([128, 512], bf16, tag="warm")
            warm_dump = pers.tile([128, 512], f32, tag="warm_dump")
            nc.vector.memset(warm_sb, 0.0)
            with tc.tile_pool(name="warm_ps", bufs=1, space="PSUM") as warm_ps:
                wps = warm_ps.tile([128, 512], f32, tag="wps")
                NWARM = 14
                for i in range(NWARM):
                    nc.tensor.matmul(
                        wps,
                        lhsT=warm_sb[:, 0:128],
                        rhs=warm_sb,
                        start=(i == 0),
                        stop=(i == NWARM - 1),
                    )
                nc.vector.tensor_copy(out=warm_dump, in_=wps)

            bq_ap = bq_d.ap()
            nc.scalar.dma_start(
                out=bq_sb,
                in_=bass.AP(tensor=bq_ap.tensor, offset=0, ap=[[1, 128], [128, EC]]),
            )

            with (
                tc.tile_pool(name="ld", bufs=1) as ld,
                tc.tile_pool(name="proj_ps", bufs=4, space="PSUM") as proj_ps,
            ):
                xT_sb = ld.tile([128, DC, S], f8, tag="xT")
                xqT_sb = ld.tile([128, DC, SQ], f8, tag="xqT")
                wk_sb = ld.tile([128, DC, D], f8, tag="wk")
                wv_sb = ld.tile([128, DC, D], f8, tag="wv")
                wq_sb = ld.tile([128, DC, D], f8, tag="wq")
                xq_sb = ld.tile([128, QT, D], f32, tag="xq")
                v_own = ld.tile([128, QT, D], f8, tag="v_own")

                # Queue balance: sync carries only the K-proj-critical xT,
                # then V staging; scalar carries wk/xqT then V readbacks;
                # gpsimd carries the rest (and the collective triggers).
                nc.sync.dma_start(out=xT_sb, in_=xT_d.ap())
                nc.scalar.dma_start(out=wk_sb, in_=wk_d.ap())
                nc.scalar.dma_start(out=xqT_sb, in_=xqT_d.ap())
                nc.gpsimd.dma_start(out=wv_sb, in_=wv_d.ap())
                nc.gpsimd.dma_start(out=wq_sb, in_=wq_d.ap())

                # kT8[e, s] = sum_d (16*Wk)[e, d] * x[s, d]  (full batch)
                for ec in range(EC):
                    for sg in range(S // 512):
                        ps = proj_ps.tile([128, 512], f32, tag="ps")
                        for i in range(NPAIR):
                            nc.tensor.matmul(
                                ps,
                                lhsT=wk_sb[:, 2 * i : 2 * i + 2, ec * 128 : (ec + 1) * 128],
                                rhs=xT_sb[:, 2 * i : 2 * i + 2, sg * 512 : (sg + 1) * 512],
                                start=(i == 0),
                                stop=(i == NPAIR - 1),
                                perf_mode=DR,
                            )
                        nc.scalar.activation(
                            out=kT8[:, ec, sg * 512 : (sg + 1) * 512],
                            in_=ps,
                            func=Act.Copy,
                        )

                # v_own[s_own, e] = sum_d x[s_own, d] * (16*Wv)[e, d]
                for st in range(QT):
                    for eg in range(D // 512):
                        ps = proj_ps.tile([128, 512], f32, tag="ps")
                        for i in range(NPAIR):
                            nc.tensor.matmul(
                                ps,
                                lhsT=xqT_sb[:, 2 * i : 2 * i + 2, st * 128 : (st + 1) * 128],
                                rhs=wv_sb[:, 2 * i : 2 * i + 2, eg * 512 : (eg + 1) * 512],
                                start=(i == 0),
                                stop=(i == NPAIR - 1),
                                perf_mode=DR,
                            )
                        nc.vector.tensor_copy(
                            out=v_own[:, st, eg * 512 : (eg + 1) * 512],
                            in_=ps,
                        )
                vx_ap = vx_in.ap()
                nc.sync.dma_start(
                    out=bass.AP(
                        tensor=vx_ap.tensor,
                        offset=0,
                        ap=[[D, 128], [128 * D, QT], [1, D]],
                    ),
                    in_=v_own,
                )
                nc.gpsimd.collective_compute(
                    "AllGather",
                    mybir.AluOpType.bypass,
                    replica_groups=GROUPS,
                    ins=[vx_in.ap().opt()],
                    outs=[vx_out.ap().opt()],
                )

                # qT8[e, q] = sum_d (16*Wq)[e, d] * x[q, d]  (+16*bq per e)
                for ec in range(EC):
                    for sg in range(SQ // 512):
                        ps = proj_ps.tile([128, 512], f32, tag="ps")
                        for i in range(NPAIR):
                            nc.tensor.matmul(
                                ps,
                                lhsT=wq_sb[:, 2 * i : 2 * i + 2, ec * 128 : (ec + 1) * 128],
                                rhs=xqT_sb[:, 2 * i : 2 * i + 2, sg * 512 : (sg + 1) * 512],
                                start=(i == 0),
                                stop=(i == NPAIR - 1),
                                perf_mode=DR,
                            )
                        nc.vector.tensor_scalar_add(
                            out=qT8[:, ec, sg * 512 : (sg + 1) * 512],
                            in0=ps,
                            scalar1=bq_sb[:, ec : ec + 1],
                        )

                # residual load — issued late so it cannot contend with the
                # critical projection inputs; only the epilogue needs it
                nc.sync.dma_start(out=xq_sb, in_=xq_d.ap())

                # V readback: both slots (uniform SPMD graph; the own slot
                # roundtrips through DRAM with identical data)
                vxo_ap = vx_out.ap()
                for r in range(2):
                    nc.scalar.dma_start(
                        out=v8[:, r * QT : (r + 1) * QT, :],
                        in_=bass.AP(
                            tensor=vxo_ap.tensor,
                            offset=r * QT * 128 * D,
                            ap=[[D, 128], [128 * D, QT], [1, D]],
                        ),
                    )

            with (
                tc.tile_pool(name="att", bufs=3) as att,
                tc.tile_pool(name="small", bufs=2) as small,
                tc.tile_pool(name="score_ps", bufs=3, space="PSUM") as score_ps,
                tc.tile_pool(name="attn_ps", bufs=3, space="PSUM") as attn_ps,
                tc.tile_pool(name="den_ps", bufs=2, space="PSUM") as den_ps,
            ):
                # scoresT[k, q] = sum_e kT8[e, k] * qT8[e, q]; exp -> PT8
                for kc in range(SC):
                    for qg in range(SQ // 512):
                        ps = score_ps.tile([128, 512], f32, tag="score")
                        for i in range(EC // 2):
                            nc.tensor.matmul(
                                ps,
                                lhsT=kT8[:, 2 * i : 2 * i + 2, kc * 128 : (kc + 1) * 128],
                                rhs=qT8[:, 2 * i : 2 * i + 2, qg * 512 : (qg + 1) * 512],
                                start=(i == 0),
                                stop=(i == EC // 2 - 1),
                                perf_mode=DR,
                            )
                        nc.scalar.activation(
                            out=PT8[:, kc, qg * 512 : (qg + 1) * 512],
                            in_=ps,
                            func=Act.Exp,
                            scale=float(SCORE_SCALE),
                            bias=nbias,
                        )

                # attn + denominators + epilogue per q-tile
                for qt in range(QT):
                    qsl = slice(qt * 128, (qt + 1) * 128)
                    dn = den_ps.tile([128, 1], f32, tag="den")
                    for j in range(SC // 2):
                        nc.tensor.matmul(
                            dn,
                            lhsT=PT8[:, 2 * j : 2 * j + 2, qsl],
                            rhs=ones2,
                            start=(j == 0),
                            stop=(j == SC // 2 - 1),
                            perf_mode=DR,
                        )
                    recip = small.tile([128, 1], f32, tag="recip", bufs=4)
                    nc.vector.reciprocal(recip, dn)
                    ot = att.tile([128, D], f32, tag="ot")
                    for j2 in range(D // 512):
                        pa = attn_ps.tile([128, 512], f32, tag="attn")
                        for j in range(SC // 2):
                            nc.tensor.matmul(
                                pa,
                                lhsT=PT8[:, 2 * j : 2 * j + 2, qsl],
                                rhs=v8[:, 2 * j : 2 * j + 2, j2 * 512 : (j2 + 1) * 512],
                                start=(j == 0),
                                stop=(j == SC // 2 - 1),
                                perf_mode=DR,
                            )
                        # out = attn * (1/(16*den)) + residual
                        nc.vector.scalar_tensor_tensor(
                            out=ot[:, j2 * 512 : (j2 + 1) * 512],
                            in0=pa,
                            scalar=recip,
                            in1=xq_sb[:, qt, j2 * 512 : (j2 + 1) * 512],
                            op0=Alu.mult,
                            op1=Alu.add,
                        )
                        nc.sync.dma_start(
                            out=out_d[qsl, j2 * 512 : (j2 + 1) * 512],
                            in_=ot[:, j2 * 512 : (j2 + 1) * 512],
                        )

    nc.compile()
    return nc


def _get_nc():
    if "nc" not in _cache:
        _cache["nc"] = _build()
    return _cache["nc"]


def _swizzle(a, np_f8):
    """[D, N] -> [128, D//128, N] partition-major, cast to fp8."""
    d, n = a.shape
    return np.ascontiguousarray(
        a.reshape(d // 128, 128, n).transpose(1, 0, 2)
    ).astype(np_f8)


def kernel(embedded, Wq, bq, Wk, bk, Wv, bv):
    import ml_dtypes

    from concourse.bass_utils import run_bass_kernel_spmd

    f8 = ml_dtypes.float8_e4m3
    x = np.ascontiguousarray(np.asarray(embedded, dtype=np.float32))
    Wq = np.asarray(Wq, dtype=np.float32)
    Wk = np.asarray(Wk, dtype=np.float32)
    Wv = np.asarray(Wv, dtype=np.float32)
    bq = np.ascontiguousarray(np.asarray(bq, dtype=np.float32))
    bv = np.ascontiguousarray(np.asarray(bv, dtype=np.float32))

    wqT = _swizzle(np.ascontiguousarray(Wq.T) * WSCALE, f8)
    wkT = _swizzle(np.ascontiguousarray(Wk.T) * WSCALE, f8)
    wvT = _swizzle(np.ascontiguousarray(Wv.T) * WSCALE, f8)
    bq16 = np.ascontiguousarray(bq * WSCALE)

    xT8 = [_swizzle(np.ascontiguousarray(x[b].T), f8) for b in range(B)]

    in_maps = []
    for c in range(NCORES):
        b, h = c // 2, c % 2
        qs = slice(h * SQ, (h + 1) * SQ)
        xh = x[b, qs, :]  # [SQ, D]
        in_maps.append(
            {
                "xT": xT8[b],
                "xqT": np.ascontiguousarray(xT8[b][:, :, qs]),
                "xq": np.ascontiguousarray(
                    (xh + bv).reshape(QT, 128, D).transpose(1, 0, 2)
                ),
                "wqT": wqT,
                "wkT": wkT,
                "wvT": wvT,
                "bq": bq16,
            }
        )

    _cache["in_maps"] = in_maps
    nc = _get_nc()
    res = run_bass_kernel_spmd(nc, in_maps, core_ids=list(range(NCORES)))
    out = np.empty((B, S, D), dtype=np.float32)
    for c in range(NCORES):
        b, h = c // 2, c % 2
        out[b, h * SQ : (h + 1) * SQ, :] = res.results[c]["out"]
    return out


# revision 12
# speedup vs baseline: 1.0350x; 1.0350x over previous
"""Distributed single-head attention block for trn2 (8 NeuronCores), fp8.

reference:
    q = x @ Wq.T + bq ; k = x @ Wk.T + bk ; v = x @ Wv.T + bv
    out = x + softmax(q @ k.T / sqrt(D)) @ v       x: [4, 2048, 1024]

Sharding: 8 cores = 4 batches x 2 query-halves. Core c owns batch c//2 and
query rows [h*1024, (h+1)*1024) with h = c%2. Each core recomputes K for
its whole batch (duplicated across the pair — cheaper than the serialized
CC-stream hop a K-exchange costs); V is projected for the own half only
and exchanged via pairwise AllGather, which hides under scoresT. A dummy
128B AllGather issued at kernel start prefetches the ~20us CC rendezvous
barrier so the real exchange starts immediately.

All matmuls run fp8e4 with DoubleRow perf mode (2x PE throughput):
weights are pre-scaled by 16 on the host so W*16 sits in the fp8 normal
range with |q,k,v| < 128 (fp8e4 max 240); the 16*16 factor on scores
folds into the exp scale, and the 16 on V folds into the softmax
reciprocal via a 16-valued ones-vector in the denominator matmul.

Scores are computed TRANSPOSED (scoresT[k,q] = K@Q^T) so the exp output
is already P^T, the lhsT the attention matmul needs — no PE transposes.
Softmax denominators (a partition-dim sum of P^T) come from a tiny
ones-vector matmul. exp uses a -4 bias (cancels between numerator and
denominator; scores reach ~7.4) to keep P far below fp8e4's 240 max.
K's bias cancels in softmax; V's bias is folded into the residual on
the host (attention weights sum to 1).
"""

import numpy as np

B, S, D = 4, 2048, 1024
SQ = S // 2  # query rows owned per core
NCORES = 8
DC = D // 128  # contraction chunks over embed
EC = D // 128  # output embed chunks
SC = S // 128  # key chunks
QT = SQ // 128  # query tiles per core
NPAIR = DC // 2  # DoubleRow pairs per 1024-deep contraction

WSCALE = 16.0  # host pre-scale on Wq/Wk/Wv (and bq); keeps |q,k,v| < 128
SCORE_SCALE = 1.0 / (np.sqrt(D) * WSCALE * WSCALE)  # 1/8192
EXP_SHIFT = 4.0  # exp(s - 4): scores reach ~7.4, so max P ~ e^3.5 << 240

_cache = {}


def _build():
    import concourse.bass as bass
    import concourse.tile as tile
    from concourse import bacc, mybir

    f32 = mybir.dt.float32
    bf16 = mybir.dt.bfloat16
    f8 = mybir.dt.float8e4
    Alu = mybir.AluOpType
    Act = mybir.ActivationFunctionType
    DR = mybir.MatmulPerfMode.DoubleRow

    nc = bacc.Bacc(None, target_bir_lowering=False, debug=False)

    xT_d = nc.declare_dram_parameter("xT", [128, DC, S], f8, isOutput=False)
    xqT_d = nc.declare_dram_parameter("xqT", [128, DC, SQ], f8, isOutput=False)
    wq_d = nc.declare_dram_parameter("wqT", [128, DC, D], f8, isOutput=False)
    wk_d = nc.declare_dram_parameter("wkT", [128, DC, D], f8, isOutput=False)
    wv_d = nc.declare_dram_parameter("wvT", [128, DC, D], f8, isOutput=False)
    bq_d = nc.declare_dram_parameter("bq", [D], f32, isOutput=False)
    xq_d = nc.declare_dram_parameter("xq", [128, QT, D], f32, isOutput=False)
    out_d = nc.declare_dram_parameter("out", [SQ, D], f32, isOutput=True)

    vx_in = nc.dram_tensor("vx_in", [QT, 128, D], f8)
    vx_out = nc.dram_tensor("vx_out", [2, QT, 128, D], f8)
    GROUPS = [[0, 1], [2, 3], [4, 5], [6, 7]]

    with tile.TileContext(nc) as tc:
        with tc.tile_pool(name="pers", bufs=1) as pers:
            qT8 = pers.tile([128, EC, SQ], f8, tag="qT8")
            kT8 = pers.tile([128, EC, S], f8, tag="kT8")
            v8 = pers.tile([128, SC, D], f8, tag="v8")
            PT8 = pers.tile([128, SC, SQ], f8, tag="PT8")
            ones2 = pers.tile([128, 2, 1], f8, tag="ones2")
            bq_sb = pers.tile([128, EC], f32, tag="bq")
            nbias = pers.tile([128, 1], f32, tag="nbias")
            nc.vector.memset(ones2, WSCALE)
            nc.vector.memset(nbias, -float(EXP_SHIFT))

            # PE warmup: dummy matmuls while the first input DMAs land, so
            # the HAM clock gate is already ramped when real work starts.
            warm_sb = pers.tile([128, 512], bf16, tag="warm")
            warm_dump = pers.tile([128, 512], f32, tag="warm_dump")
            nc.vector.memset(warm_sb, 0.0)
            with tc.tile_pool(name="warm_ps", bufs=1, space="PSUM") as warm_ps:
                wps = warm_ps.tile([128, 512], f32, tag="wps")
                NWARM = 14
                for i in range(NWARM):
                    nc.tensor.matmul(
                        wps,
                        lhsT=warm_sb[:, 0:128],
                        rhs=warm_sb,
                        start=(i == 0),
                        stop=(i == NWARM - 1),
                    )
                nc.vector.tensor_copy(out=warm_dump, in_=wps)

            bq_ap = bq_d.ap()
            nc.scalar.dma_start(
                out=bq_sb,
                in_=bass.AP(tensor=bq_ap.tensor, offset=0, ap=[[1, 128], [128, EC]]),
            )

            with (
                tc.tile_pool(name="ld", bufs=1) as ld,
                tc.tile_pool(name="proj_ps", bufs=4, space="PSUM") as proj_ps,
            ):
                xT_sb = ld.tile([128, DC, S], f8, tag="xT")
                xqT_sb = ld.tile([128, DC, SQ], f8, tag="xqT")
                wk_sb = ld.tile([128, DC, D], f8, tag="wk")
                wv_sb = ld.tile([128, DC, D], f8, tag="wv")
                wq_sb = ld.tile([128, DC, D], f8, tag="wq")
                xq_sb = ld.tile([128, QT, D], f32, tag="xq")
                v_own = ld.tile([128, QT, D], f8, tag="v_own")

                # Each engine owns one ~90GB/s dynamic DMA queue, so the
                # K-proj-critical 3MB (xT + wk) is split across all three;
                # the second wave (xqT/wv/wq) follows on the same queues.
                nc.sync.dma_start(
                    out=xT_sb[:, : DC // 2, :], in_=xT_d[:, : DC // 2, :]
                )
                nc.scalar.dma_start(
                    out=xT_sb[:, DC // 2 :, :], in_=xT_d[:, DC // 2 :, :]
                )
                nc.gpsimd.dma_start(out=wk_sb, in_=wk_d.ap())
                nc.sync.dma_start(out=xqT_sb, in_=xqT_d.ap())
                nc.scalar.dma_start(out=wv_sb, in_=wv_d.ap())
                nc.gpsimd.dma_start(out=wq_sb, in_=wq_d.ap())

                # kT8[e, s] = sum_d (16*Wk)[e, d] * x[s, d]  (full batch)
                for ec in range(EC):
                    for sg in range(S // 512):
                        ps = proj_ps.tile([128, 512], f32, tag="ps")
                        for i in range(NPAIR):
                            nc.tensor.matmul(
                                ps,
                                lhsT=wk_sb[:, 2 * i : 2 * i + 2, ec * 128 : (ec + 1) * 128],
                                rhs=xT_sb[:, 2 * i : 2 * i + 2, sg * 512 : (sg + 1) * 512],
                                start=(i == 0),
                                stop=(i == NPAIR - 1),
                                perf_mode=DR,
                            )
                        nc.scalar.activation(
                            out=kT8[:, ec, sg * 512 : (sg + 1) * 512],
                            in_=ps,
                            func=Act.Copy,
                        )

                # v_own[s_own, e] = sum_d x[s_own, d] * (16*Wv)[e, d]
                for st in range(QT):
                    for eg in range(D // 512):
                        ps = proj_ps.tile([128, 512], f32, tag="ps")
                        for i in range(NPAIR):
                            nc.tensor.matmul(
                                ps,
                                lhsT=xqT_sb[:, 2 * i : 2 * i + 2, st * 128 : (st + 1) * 128],
                                rhs=wv_sb[:, 2 * i : 2 * i + 2, eg * 512 : (eg + 1) * 512],
                                start=(i == 0),
                                stop=(i == NPAIR - 1),
                                perf_mode=DR,
                            )
                        nc.vector.tensor_copy(
                            out=v_own[:, st, eg * 512 : (eg + 1) * 512],
                            in_=ps,
                        )
                vx_ap = vx_in.ap()
                HQ = QT // 2
                nc.sync.dma_start(
                    out=bass.AP(
                        tensor=vx_ap.tensor,
                        offset=0,
                        ap=[[D, 128], [128 * D, HQ], [1, D]],
                    ),
                    in_=v_own[:, :HQ, :],
                )
                nc.gpsimd.dma_start(
                    out=bass.AP(
                        tensor=vx_ap.tensor,
                        offset=HQ * 128 * D,
                        ap=[[D, 128], [128 * D, HQ], [1, D]],
                    ),
                    in_=v_own[:, HQ:, :],
                )
                nc.gpsimd.collective_compute(
                    "AllGather",
                    mybir.AluOpType.bypass,
                    replica_groups=GROUPS,
                    ins=[vx_in.ap().opt()],
                    outs=[vx_out.ap().opt()],
                )

                # qT8[e, q] = sum_d (16*Wq)[e, d] * x[q, d]  (+16*bq per e)
                for ec in range(EC):
                    for sg in range(SQ // 512):
                        ps = proj_ps.tile([128, 512], f32, tag="ps")
                        for i in range(NPAIR):
                            nc.tensor.matmul(
                                ps,
                                lhsT=wq_sb[:, 2 * i : 2 * i + 2, ec * 128 : (ec + 1) * 128],
                                rhs=xqT_sb[:, 2 * i : 2 * i + 2, sg * 512 : (sg + 1) * 512],
                                start=(i == 0),
                                stop=(i == NPAIR - 1),
                                perf_mode=DR,
                            )
                        nc.vector.tensor_scalar_add(
                            out=qT8[:, ec, sg * 512 : (sg + 1) * 512],
                            in0=ps,
                            scalar1=bq_sb[:, ec : ec + 1],
                        )

                # residual load — issued late so it cannot contend with the
                # critical projection inputs; only the epilogue needs it
                nc.gpsimd.dma_start(out=xq_sb, in_=xq_d.ap())

                # V readback: both slots (uniform SPMD graph; the own slot
                # roundtrips through DRAM with identical data)
                vxo_ap = vx_out.ap()
                for r, eng in ((0, nc.scalar), (1, nc.sync)):
                    eng.dma_start(
                        out=v8[:, r * QT : (r + 1) * QT, :],
                        in_=bass.AP(
                            tensor=vxo_ap.tensor,
                            offset=r * QT * 128 * D,
                            ap=[[D, 128], [128 * D, QT], [1, D]],
                        ),
                    )

            with (
                tc.tile_pool(name="att", bufs=3) as att,
                tc.tile_pool(name="small", bufs=2) as small,
                tc.tile_pool(name="score_ps", bufs=3, space="PSUM") as score_ps,
                tc.tile_pool(name="attn_ps", bufs=3, space="PSUM") as attn_ps,
                tc.tile_pool(name="den_ps", bufs=2, space="PSUM") as den_ps,
            ):
                # scoresT[k, q] = sum_e kT8[e, k] * qT8[e, q]; exp -> PT8
                for kc in range(SC):
                    for qg in range(SQ // 512):
                        ps = score_ps.tile([128, 512], f32, tag="score")
                        for i in range(EC // 2):
                            nc.tensor.matmul(
                                ps,
                                lhsT=kT8[:, 2 * i : 2 * i + 2, kc * 128 : (kc + 1) * 128],
                                rhs=qT8[:, 2 * i : 2 * i + 2, qg * 512 : (qg + 1) * 512],
                                start=(i == 0),
                                stop=(i == EC // 2 - 1),
                                perf_mode=DR,
                            )
                        nc.scalar.activation(
                            out=PT8[:, kc, qg * 512 : (qg + 1) * 512],
                            in_=ps,
                            func=Act.Exp,
                            scale=float(SCORE_SCALE),
                            bias=nbias,
                        )

                # attn + denominators + epilogue per q-tile
                for qt in range(QT):
                    qsl = slice(qt * 128, (qt + 1) * 128)
                    dn = den_ps.tile([128, 1], f32, tag="den")
                    for j in range(SC // 2):
                        nc.tensor.matmul(
                            dn,
                            lhsT=PT8[:, 2 * j : 2 * j + 2, qsl],
                            rhs=ones2,
                            start=(j == 0),
                            stop=(j == SC // 2 - 1),
                            perf_mode=DR,
                        )
                    recip = small.tile([128, 1], f32, tag="recip", bufs=4)
                    nc.vector.reciprocal(recip, dn)
                    ot = att.tile([128, D], f32, tag="ot")
                    for j2 in range(D // 512):
                        pa = attn_ps.tile([128, 512], f32, tag="attn")
                        for j in range(SC // 2):
                            nc.tensor.matmul(
                                pa,
                                lhsT=PT8[:, 2 * j : 2 * j + 2, qsl],
                                rhs=v8[:, 2 * j : 2 * j + 2, j2 * 512 : (j2 + 1) * 512],
                                start=(j == 0),
                                stop=(j == SC // 2 - 1),
                                perf_mode=DR,
                            )
                        # out = attn * (1/(16*den)) + residual
                        nc.vector.scalar_tensor_tensor(
                            out=ot[:, j2 * 512 : (j2 + 1) * 512],
                            in0=pa,
                            scalar=recip,
                            in1=xq_sb[:, qt, j2 * 512 : (j2 + 1) * 512],
                            op0=Alu.mult,
                            op1=Alu.add,
                        )
                        eng = (nc.sync, nc.scalar, nc.gpsimd)[(qt * 2 + j2) % 3]
                        eng.dma_start(
                            out=out_d[qsl, j2 * 512 : (j2 + 1) * 512],
                            in_=ot[:, j2 * 512 : (j2 + 1) * 512],
                        )

    nc.compile()
    return nc


def _get_nc():
    if "nc" not in _cache:
        _cache["nc"] = _build()
    return _cache["nc"]


def _swizzle(a, np_f8):
    """[D, N] -> [128, D//128, N] partition-major, cast to fp8."""
    d, n = a.shape
    return np.ascontiguousarray(
        a.reshape(d // 128, 128, n).transpose(1, 0, 2)
    ).astype(np_f8)


def kernel(embedded, Wq, bq, Wk, bk, Wv, bv):
    import ml_dtypes

    from concourse.bass_utils import run_bass_kernel_spmd

    f8 = ml_dtypes.float8_e4m3
    x = np.ascontiguousarray(np.asarray(embedded, dtype=np.float32))
    Wq = np.asarray(Wq, dtype=np.float32)
    Wk = np.asarray(Wk, dtype=np.float32)
    Wv = np.asarray(Wv, dtype=np.float32)
    bq = np.ascontiguousarray(np.asarray(bq, dtype=np.float32))
    bv = np.ascontiguousarray(np.asarray(bv, dtype=np.float32))

    wqT = _swizzle(np.ascontiguousarray(Wq.T) * WSCALE, f8)
    wkT = _swizzle(np.ascontiguousarray(Wk.T) * WSCALE, f8)
    wvT = _swizzle(np.ascontiguousarray(Wv.T) * WSCALE, f8)
    bq16 = np.ascontiguousarray(bq * WSCALE)

    xT8 = [_swizzle(np.ascontiguousarray(x[b].T), f8) for b in range(B)]

    in_maps = []
    for c in range(NCORES):
        b, h = c // 2, c % 2
        qs = slice(h * SQ, (h + 1) * SQ)
        xh = x[b, qs, :]  # [SQ, D]
        in_maps.append(
            {
                "xT": xT8[b],
                "xqT": np.ascontiguousarray(xT8[b][:, :, qs]),
                "xq": np.ascontiguousarray(
                    (xh + bv).reshape(QT, 128, D).transpose(1, 0, 2)
                ),
                "wqT": wqT,
                "wkT": wkT,
                "wvT": wvT,
                "bq": bq16,
            }
        )

    _cache["in_maps"] = in_maps
    nc = _get_nc()
    res = run_bass_kernel_spmd(nc, in_maps, core_ids=list(range(NCORES)))
    out = np.empty((B, S, D), dtype=np.float32)
    for c in range(NCORES):
        b, h = c // 2, c % 2
        out[b, h * SQ : (h + 1) * SQ, :] = res.results[c]["out"]
    return out


# revision 13
# speedup vs baseline: 1.0987x; 1.0615x over previous
"""Distributed single-head attention block for trn2 (8 NeuronCores), fp8.

reference:
    q = x @ Wq.T + bq ; k = x @ Wk.T + bk ; v = x @ Wv.T + bv
    out = x + softmax(q @ k.T / sqrt(D)) @ v       x: [4, 2048, 1024]

Sharding: 8 cores = 4 batches x 2 query-halves. Core c owns batch c//2 and
query rows [h*1024, (h+1)*1024) with h = c%2. Each core recomputes K for
its whole batch (duplicated across the pair — cheaper than the serialized
CC-stream hop a K-exchange costs); V is projected for the own half only
and exchanged via pairwise AllGather, which hides under scoresT. A dummy
128B AllGather issued at kernel start prefetches the ~20us CC rendezvous
barrier so the real exchange starts immediately.

All matmuls run fp8e4 with DoubleRow perf mode (2x PE throughput):
weights are pre-scaled by 16 on the host so W*16 sits in the fp8 normal
range with |q,k,v| < 128 (fp8e4 max 240); the 16*16 factor on scores
folds into the exp scale, and the 16 on V folds into the softmax
reciprocal via a 16-valued ones-vector in the denominator matmul.

Scores are computed TRANSPOSED (scoresT[k,q] = K@Q^T) so the exp output
is already P^T, the lhsT the attention matmul needs — no PE transposes.
Softmax denominators (a partition-dim sum of P^T) come from a tiny
ones-vector matmul. exp uses a -4 bias (cancels between numerator and
denominator; scores reach ~7.4) to keep P far below fp8e4's 240 max.
K's bias cancels in softmax; V's bias is folded into the residual on
the host (attention weights sum to 1).
"""

import numpy as np

B, S, D = 4, 2048, 1024
SQ = S // 2  # query rows owned per core
NCORES = 8
DC = D // 128  # contraction chunks over embed
EC = D // 128  # output embed chunks
SC = S // 128  # key chunks
QT = SQ // 128  # query tiles per core
NPAIR = DC // 2  # DoubleRow pairs per 1024-deep contraction

WSCALE = 16.0  # host pre-scale on Wq/Wk/Wv (and bq); keeps |q,k,v| < 128
SCORE_SCALE = 1.0 / (np.sqrt(D) * WSCALE * WSCALE)  # 1/8192
EXP_SHIFT = 4.0  # exp(s - 4): scores reach ~7.4, so max P ~ e^3.5 << 240

_cache = {}


def _build():
    import concourse.bass as bass
    import concourse.tile as tile
    from concourse import bacc, mybir

    f32 = mybir.dt.float32
    bf16 = mybir.dt.bfloat16
    f8 = mybir.dt.float8e4
    Alu = mybir.AluOpType
    Act = mybir.ActivationFunctionType
    DR = mybir.MatmulPerfMode.DoubleRow

    nc = bacc.Bacc(None, target_bir_lowering=False, debug=False)

    xT_d = nc.declare_dram_parameter("xT", [128, DC, S], f8, isOutput=False)
    xqT_d = nc.declare_dram_parameter("xqT", [128, DC, SQ], f8, isOutput=False)
    wq_d = nc.declare_dram_parameter("wqT", [128, DC, D], f8, isOutput=False)
    wk_d = nc.declare_dram_parameter("wkT", [128, DC, D], f8, isOutput=False)
    wv_d = nc.declare_dram_parameter("wvT", [128, DC, D], f8, isOutput=False)
    bq_d = nc.declare_dram_parameter("bq", [D], f32, isOutput=False)
    xq_d = nc.declare_dram_parameter("xq", [128, QT, D], f32, isOutput=False)
    out_d = nc.declare_dram_parameter("out", [SQ, D], f32, isOutput=True)

    vx_in = nc.dram_tensor("vx_in", [QT, 128, D], f8)
    vx_out = nc.dram_tensor("vx_out", [2, QT, 128, D], f8)
    GROUPS = [[0, 1], [2, 3], [4, 5], [6, 7]]

    with tile.TileContext(nc) as tc:
        with tc.tile_pool(name="pers", bufs=1) as pers:
            qT8 = pers.tile([128, EC, SQ], f8, tag="qT8")
            kT8 = pers.tile([128, EC, S], f8, tag="kT8")
            v8 = pers.tile([128, SC, D], f8, tag="v8")
            PT8 = pers.tile([128, SC, SQ], f8, tag="PT8")
            ones2 = pers.tile([128, 2, 1], f8, tag="ones2")
            bq_sb = pers.tile([128, EC], f32, tag="bq")
            nbias = pers.tile([128, 1], f32, tag="nbias")
            nc.vector.memset(ones2, WSCALE)
            nc.vector.memset(nbias, -float(EXP_SHIFT))

            # PE warmup: dummy matmuls while the first input DMAs land, so
            # the HAM clock gate is already ramped when real work starts.
            warm_sb = pers.tile([128, 512], bf16, tag="warm")
            warm_dump = pers.tile([128, 512], f32, tag="warm_dump")
            nc.vector.memset(warm_sb, 0.0)
            with tc.tile_pool(name="warm_ps", bufs=1, space="PSUM") as warm_ps:
                wps = warm_ps.tile([128, 512], f32, tag="wps")
                NWARM = 14
                for i in range(NWARM):
                    nc.tensor.matmul(
                        wps,
                        lhsT=warm_sb[:, 0:128],
                        rhs=warm_sb,
                        start=(i == 0),
                        stop=(i == NWARM - 1),
                    )
                nc.vector.tensor_copy(out=warm_dump, in_=wps)

            bq_ap = bq_d.ap()
            nc.scalar.dma_start(
                out=bq_sb,
                in_=bass.AP(tensor=bq_ap.tensor, offset=0, ap=[[1, 128], [128, EC]]),
            )

            with (
                tc.tile_pool(name="ld", bufs=1) as ld,
                tc.tile_pool(name="proj_ps", bufs=4, space="PSUM") as proj_ps,
            ):
                xT_sb = ld.tile([128, DC, S], f8, tag="xT")
                xqT_sb = ld.tile([128, DC, SQ], f8, tag="xqT")
                wk_sb = ld.tile([128, DC, D], f8, tag="wk")
                wv_sb = ld.tile([128, DC, D], f8, tag="wv")
                wq_sb = ld.tile([128, DC, D], f8, tag="wq")
                xq_sb = ld.tile([128, QT, D], f32, tag="xq")
                v_own = ld.tile([128, QT, D], f8, tag="v_own")

                # HBM is the shared constraint (~360GB/s): wave 1 loads only
                # what V proj needs (2MB), wave 2 the K inputs, wq last; the
                # 4MB residual is issued AFTER the kT copies on scalar so its
                # transfer cannot overlap the critical input window.
                nc.sync.dma_start(out=xqT_sb, in_=xqT_d.ap())
                nc.scalar.dma_start(out=wv_sb, in_=wv_d.ap())
                nc.gpsimd.dma_start(out=wk_sb, in_=wk_d.ap())
                nc.sync.dma_start(
                    out=xT_sb[:, : DC // 2, :], in_=xT_d[:, : DC // 2, :]
                )
                nc.scalar.dma_start(
                    out=xT_sb[:, DC // 2 :, :], in_=xT_d[:, DC // 2 :, :]
                )
                nc.gpsimd.dma_start(out=wq_sb, in_=wq_d.ap())

                # v_own[s_own, e] = sum_d x[s_own, d] * (16*Wv)[e, d]
                for st in range(QT):
                    for eg in range(D // 512):
                        ps = proj_ps.tile([128, 512], f32, tag="ps")
                        for i in range(NPAIR):
                            nc.tensor.matmul(
                                ps,
                                lhsT=xqT_sb[:, 2 * i : 2 * i + 2, st * 128 : (st + 1) * 128],
                                rhs=wv_sb[:, 2 * i : 2 * i + 2, eg * 512 : (eg + 1) * 512],
                                start=(i == 0),
                                stop=(i == NPAIR - 1),
                                perf_mode=DR,
                            )
                        nc.vector.tensor_copy(
                            out=v_own[:, st, eg * 512 : (eg + 1) * 512],
                            in_=ps,
                        )
                vx_ap = vx_in.ap()
                HQ = QT // 2
                nc.sync.dma_start(
                    out=bass.AP(
                        tensor=vx_ap.tensor,
                        offset=0,
                        ap=[[D, 128], [128 * D, HQ], [1, D]],
                    ),
                    in_=v_own[:, :HQ, :],
                )
                nc.gpsimd.dma_start(
                    out=bass.AP(
                        tensor=vx_ap.tensor,
                        offset=HQ * 128 * D,
                        ap=[[D, 128], [128 * D, HQ], [1, D]],
                    ),
                    in_=v_own[:, HQ:, :],
                )
                nc.gpsimd.collective_compute(
                    "AllGather",
                    mybir.AluOpType.bypass,
                    replica_groups=GROUPS,
                    ins=[vx_in.ap().opt()],
                    outs=[vx_out.ap().opt()],
                )

                # kT8[e, s] = sum_d (16*Wk)[e, d] * x[s, d]  (full batch)
                for ec in range(EC):
                    for sg in range(S // 512):
                        ps = proj_ps.tile([128, 512], f32, tag="ps")
                        for i in range(NPAIR):
                            nc.tensor.matmul(
                                ps,
                                lhsT=wk_sb[:, 2 * i : 2 * i + 2, ec * 128 : (ec + 1) * 128],
                                rhs=xT_sb[:, 2 * i : 2 * i + 2, sg * 512 : (sg + 1) * 512],
                                start=(i == 0),
                                stop=(i == NPAIR - 1),
                                perf_mode=DR,
                            )
                        nc.scalar.activation(
                            out=kT8[:, ec, sg * 512 : (sg + 1) * 512],
                            in_=ps,
                            func=Act.Copy,
                        )

                # residual load: issued on scalar after the kT copies, so the
                # 4MB transfer starts only once the critical inputs are in
                nc.scalar.dma_start(out=xq_sb, in_=xq_d.ap())

                # qT8[e, q] = sum_d (16*Wq)[e, d] * x[q, d]  (+16*bq per e)
                for ec in range(EC):
                    for sg in range(SQ // 512):
                        ps = proj_ps.tile([128, 512], f32, tag="ps")
                        for i in range(NPAIR):
                            nc.tensor.matmul(
                                ps,
                                lhsT=wq_sb[:, 2 * i : 2 * i + 2, ec * 128 : (ec + 1) * 128],
                                rhs=xqT_sb[:, 2 * i : 2 * i + 2, sg * 512 : (sg + 1) * 512],
                                start=(i == 0),
                                stop=(i == NPAIR - 1),
                                perf_mode=DR,
                            )
                        nc.vector.tensor_scalar_add(
                            out=qT8[:, ec, sg * 512 : (sg + 1) * 512],
                            in0=ps,
                            scalar1=bq_sb[:, ec : ec + 1],
                        )

                # V readback: both slots (uniform SPMD graph; the own slot
                # roundtrips through DRAM with identical data)
                vxo_ap = vx_out.ap()
                for r, eng in ((0, nc.scalar), (1, nc.sync)):
                    eng.dma_start(
                        out=v8[:, r * QT : (r + 1) * QT, :],
                        in_=bass.AP(
                            tensor=vxo_ap.tensor,
                            offset=r * QT * 128 * D,
                            ap=[[D, 128], [128 * D, QT], [1, D]],
                        ),
                    )

            with (
                tc.tile_pool(name="att", bufs=3) as att,
                tc.tile_pool(name="small", bufs=2) as small,
                tc.tile_pool(name="score_ps", bufs=3, space="PSUM") as score_ps,
                tc.tile_pool(name="attn_ps", bufs=3, space="PSUM") as attn_ps,
                tc.tile_pool(name="den_ps", bufs=2, space="PSUM") as den_ps,
            ):
                # scoresT[k, q] = sum_e kT8[e, k] * qT8[e, q]; exp -> PT8
                for kc in range(SC):
                    for qg in range(SQ // 512):
                        ps = score_ps.tile([128, 512], f32, tag="score")
                        for i in range(EC // 2):
                            nc.tensor.matmul(
                                ps,
                                lhsT=kT8[:, 2 * i : 2 * i + 2, kc * 128 : (kc + 1) * 128],
                                rhs=qT8[:, 2 * i : 2 * i + 2, qg * 512 : (qg + 1) * 512],
                                start=(i == 0),
                                stop=(i == EC // 2 - 1),
                                perf_mode=DR,
                            )
                        nc.scalar.activation(
                            out=PT8[:, kc, qg * 512 : (qg + 1) * 512],
                            in_=ps,
                            func=Act.Exp,
                            scale=float(SCORE_SCALE),
                            bias=nbias,
                        )

                # attn + denominators + epilogue per q-tile
                for qt in range(QT):
                    qsl = slice(qt * 128, (qt + 1) * 128)
                    dn = den_ps.tile([128, 1], f32, tag="den")
                    for j in range(SC // 2):
                        nc.tensor.matmul(
                            dn,
                            lhsT=PT8[:, 2 * j : 2 * j + 2, qsl],
                            rhs=ones2,
                            start=(j == 0),
                            stop=(j == SC // 2 - 1),
                            perf_mode=DR,
                        )
                    recip = small.tile([128, 1], f32, tag="recip", bufs=4)
                    nc.vector.reciprocal(recip, dn)
                    ot = att.tile([128, D], f32, tag="ot")
                    for j2 in range(D // 512):
                        pa = attn_ps.tile([128, 512], f32, tag="attn")
                        for j in range(SC // 2):
                            nc.tensor.matmul(
                                pa,
                                lhsT=PT8[:, 2 * j : 2 * j + 2, qsl],
                                rhs=v8[:, 2 * j : 2 * j + 2, j2 * 512 : (j2 + 1) * 512],
                                start=(j == 0),
                                stop=(j == SC // 2 - 1),
                                perf_mode=DR,
                            )
                        # out = attn * (1/(16*den)) + residual
                        nc.vector.scalar_tensor_tensor(
                            out=ot[:, j2 * 512 : (j2 + 1) * 512],
                            in0=pa,
                            scalar=recip,
                            in1=xq_sb[:, qt, j2 * 512 : (j2 + 1) * 512],
                            op0=Alu.mult,
                            op1=Alu.add,
                        )
                        eng = (nc.sync, nc.scalar, nc.gpsimd)[(qt * 2 + j2) % 3]
                        eng.dma_start(
                            out=out_d[qsl, j2 * 512 : (j2 + 1) * 512],
                            in_=ot[:, j2 * 512 : (j2 + 1) * 512],
                        )

    nc.compile()
    return nc


def _get_nc():
    if "nc" not in _cache:
        _cache["nc"] = _build()
    return _cache["nc"]


def _swizzle(a, np_f8):
    """[D, N] -> [128, D//128, N] partition-major, cast to fp8."""
    d, n = a.shape
    return np.ascontiguousarray(
        a.reshape(d // 128, 128, n).transpose(1, 0, 2)
    ).astype(np_f8)


def kernel(embedded, Wq, bq, Wk, bk, Wv, bv):
    import ml_dtypes

    from concourse.bass_utils import run_bass_kernel_spmd

    f8 = ml_dtypes.float8_e4m3
    x = np.ascontiguousarray(np.asarray(embedded, dtype=np.float32))
    Wq = np.asarray(Wq, dtype=np.float32)
    Wk = np.asarray(Wk, dtype=np.float32)
    Wv = np.asarray(Wv, dtype=np.float32)
    bq = np.ascontiguousarray(np.asarray(bq, dtype=np.float32))
    bv = np.ascontiguousarray(np.asarray(bv, dtype=np.float32))

    wqT = _swizzle(np.ascontiguousarray(Wq.T) * WSCALE, f8)
    wkT = _swizzle(np.ascontiguousarray(Wk.T) * WSCALE, f8)
    wvT = _swizzle(np.ascontiguousarray(Wv.T) * WSCALE, f8)
    bq16 = np.ascontiguousarray(bq * WSCALE)

    xT8 = [_swizzle(np.ascontiguousarray(x[b].T), f8) for b in range(B)]

    in_maps = []
    for c in range(NCORES):
        b, h = c // 2, c % 2
        qs = slice(h * SQ, (h + 1) * SQ)
        xh = x[b, qs, :]  # [SQ, D]
        in_maps.append(
            {
                "xT": xT8[b],
                "xqT": np.ascontiguousarray(xT8[b][:, :, qs]),
                "xq": np.ascontiguousarray(
                    (xh + bv).reshape(QT, 128, D).transpose(1, 0, 2)
                ),
                "wqT": wqT,
                "wkT": wkT,
                "wvT": wvT,
                "bq": bq16,
            }
        )

    _cache["in_maps"] = in_maps
    nc = _get_nc()
    res = run_bass_kernel_spmd(nc, in_maps, core_ids=list(range(NCORES)))
    out = np.empty((B, S, D), dtype=np.float32)
    for c in range(NCORES):
        b, h = c // 2, c % 2
        out[b, h * SQ : (h + 1) * SQ, :] = res.results[c]["out"]
    return out
